# revision 12
# baseline (speedup 1.0000x reference)
"""Trainium2 Bass kernel for CartNN minimal-NEAT forward pass.

Computes out = tanh(tanh(x @ w + b))[:, None] for x [16384, 4096] f32,
w [4096] f32, b [1] f32, data-parallel across 8 NeuronCores (2048 batch
rows per core). Memory-bound: each core streams its 32 MiB x shard once.

Per-core structure (measured on HW, iterated via NTFF profiles):
  - x streams as 16 [128, 4096] tiles split alternately across the two
    physical HWDGE rings (even tiles sync/qSPDynamicHW, odd tiles
    scalar/qActDynamicHW). All x DMAs write full 128-partition tiles:
    partition-sliced destinations (e.g. [0:120]) measurably halve the
    per-descriptor SDMA rate (port-swizzle misalignment), which is why
    engine-15 starvation via 120-row tiles regressed 117 -> 175 us.
  - SDMA engine 15 is ~20% slower than engines 0-14 for sync-ring
    traffic (known HW quirk), and with uniform tiles it carries 1/16
    of the bytes, so an all-sync stream is engine-15-bound: ~98 us
    busy vs ~79.5 for the rest. The ring split tests whether the
    slowdown is per-ring (descriptor-fetch port contention).
  - w is loaded once (16 KiB, scalar ring) and broadcast to all 128
    partitions by TensorE outer products ones[128,1] @ w[1,512]
    (PSUM->SBUF copies on ScalarE): zero extra HBM traffic and no
    sync-ring involvement (stride-0 DRAM broadcast DMAs measurably
    poison the x stream).
  - The dot product is one fused mul+reduce VectorE op per tile
    (affine_mul_reduce, ~4.5 us; TensorTensorReduce crashes the
    device). The first 4 tiles are split along K with staggered
    emission so DVE starts before the w broadcast completes; their
    quarter-partials are folded into acc right after the stagger (DVE
    program order) so the mid-stream output chunk doesn't wait.
  - Tiles 8 and 10 are offloaded off VectorE: GpSimd multiplies,
    ScalarE reduces via activation-accum. With the ~6 us/tile
    engine-15-paced arrival rate DVE then idles between tiles instead
    of being backlogged when the last x bytes land.
  - Output is emitted in two chunks: tiles 0..13 go tanh(tanh(.+b)) ->
    TensorE transpose -> [14,128] DMA mid-stream (fully hidden under
    the x stream tail); the end-of-kernel chain is only tiles 14/15:
    final quarter affines, 3 adds, tanh x2 on [128,2], transpose, one
    1 KiB DMA of 512-B rows from the scalar ring (ScalarE just wrote
    the data, skipping the ScalarE->Sync semaphore hop).
  - The last two tiles are split (loads AND compute: halves for t=14,
    quarters for t=15) so the final compute piece starts on the last
    512 KiB rather than the last 2 MiB.
"""

import numpy as np

import concourse.bacc as bacc
import concourse.mybir as mybir
from concourse.bass_utils import run_bass_kernel_spmd
from concourse.masks import make_identity
from concourse.tile import TileContext

N_CORES = 8
BATCH = 16384
IN_SIZE = 4096
P = 128
B_PER_CORE = BATCH // N_CORES  # 2048
N_TILES = B_PER_CORE // P  # 16

_NC_CACHE = None


def _build():
    nc = bacc.Bacc(
        "TRN2",
        target_bir_lowering=False,
        debug=False,
        num_devices=N_CORES,
    )
    x = nc.dram_tensor(
        "x", [B_PER_CORE, IN_SIZE], mybir.dt.float32, kind="ExternalInput"
    )
    w = nc.dram_tensor("w", [IN_SIZE], mybir.dt.float32, kind="ExternalInput")
    b = nc.dram_tensor("b", [1], mybir.dt.float32, kind="ExternalInput")
    y = nc.dram_tensor("y", [B_PER_CORE, 1], mybir.dt.float32, kind="ExternalOutput")

    xt = x.rearrange("(t p) k -> t p k", p=P)  # [16, 128, 4096]
    yv = y.rearrange("(t p) o -> t (p o)", p=P)  # [16, 128], 512B rows

    N_A = 15  # tiles 0..14 emitted mid-stream; tile 15 at the end
    N_B = N_TILES - N_A

    with TileContext(nc) as tc:
        with (
            tc.tile_pool(name="xpool", bufs=8) as xpool,
            tc.tile_pool(name="scratch", bufs=1) as spool,
            tc.tile_pool(name="consts", bufs=1) as cpool,
            tc.tile_pool(name="psum", bufs=1, space="PSUM") as ppool,
        ):
            x_tiles = {}

            def ring(t):
                return nc.sync if t % 2 == 0 else nc.scalar

            def load_x(t):
                x_PK = xpool.tile([P, IN_SIZE], mybir.dt.float32)
                ring(t).dma_start(out=x_PK[:], in_=xt[t])
                x_tiles[t] = x_PK

            # Tile 0's load is the very first sync-ring op so the x
            # stream starts as early as the NEFF preamble allows.
            load_x(0)

            # w/b arrive on the scalar HWDGE ring, keeping the sync ring
            # clear for the stream. TensorE broadcasts w to all 128
            # partitions chunk by chunk: ones[128,1] @ w[1,512] outer
            # products, copied PSUM->SBUF by the otherwise-idle ScalarE.
            w_1K = cpool.tile([1, IN_SIZE], mybir.dt.float32)
            nc.scalar.dma_start(out=w_1K[:], in_=w[None, :])
            b_11 = cpool.tile([1, 1], mybir.dt.float32)
            nc.scalar.dma_start(out=b_11[:], in_=b[None, :])
            ones_1P = cpool.tile([1, P], mybir.dt.float32)
            nc.vector.memset(ones_1P[:], 1.0)

            acc_PT = cpool.tile([P, N_TILES], mybir.dt.float32)
            NSPLIT = 4
            NQT = 4  # tiles that use the quarter-split
            STAGGER = 3
            KQ = IN_SIZE // NSPLIT
            accs_q = [
                cpool.tile([P, NQT], mybir.dt.float32, name=f"acc_{q}")
                for q in range(1, NSPLIT)
            ]
            acc_last = cpool.tile([P, 4], mybir.dt.float32)

            w_PK = cpool.tile([P, IN_SIZE], mybir.dt.float32)
            NCHUNK = 512
            for c in range(IN_SIZE // NCHUNK):
                cs = slice(c * NCHUNK, (c + 1) * NCHUNK)
                w_psum = ppool.tile([P, NCHUNK], mybir.dt.float32, bufs=2)
                nc.tensor.matmul(w_psum[:], ones_1P[:], w_1K[0:1, cs])
                nc.scalar.copy(w_PK[:, cs], w_psum[:])
            b_psum = ppool.tile([P, 1], mybir.dt.float32)
            nc.tensor.matmul(b_psum[:], ones_1P[:], b_11[:])
            b_P1 = cpool.tile([P, 1], mybir.dt.float32)
            nc.scalar.copy(b_P1[:], b_psum[:])
            ident = cpool.tile([P, P], mybir.dt.float32)
            make_identity(nc, ident[:])

            prod_PK = spool.tile([P, IN_SIZE], mybir.dt.float32)

            # The first 4 tiles are split into quarter-K ops with a
            # staggered emission (quarter q of tile t at step t + 3q):
            # quarter q only needs w[q*1024:(q+1)*1024], so DVE starts as
            # soon as the first w chunks are broadcast instead of waiting
            # for all of w. The Tile scheduler keeps same-engine program
            # order, so the stagger must be explicit.
            def emit_quarter(t, q):
                seg = slice(q * KQ, (q + 1) * KQ)
                acc = acc_PT[:, t : t + 1] if q == 0 else accs_q[q - 1][:, t : t + 1]
                nc.vector.affine_mul_reduce(
                    out=prod_PK[:, seg],
                    accum_out=acc,
                    in0=x_tiles[t][:, seg],
                    in1=w_PK[:, seg],
                    scale=1.0,
                    bias=0.0,
                )

            for i in range(NQT + STAGGER * (NSPLIT - 1)):
                if 0 < i < NQT:
                    load_x(i)
                if i < NQT:
                    emit_quarter(i, 0)
                for q in range(1, NSPLIT):
                    t = i - STAGGER * q
                    if 0 <= t < NQT:
                        emit_quarter(t, q)
            # Fold the quarter partials early (DVE program order!) so the
            # mid-stream chunk-A output only waits on tile 13's affine.
            for acc_q in accs_q:
                nc.vector.tensor_add(acc_PT[:, 0:NQT], acc_PT[:, 0:NQT], acc_q[:])

            # Mid tiles. Two are offloaded off the (binding) VectorE:
            # GpSimd does the elementwise multiply, ScalarE reduces it via
            # activation-accum. Both engines are otherwise idle mid-kernel
            # and finish long before their results are needed. The
            # offloaded tiles MUST be >= 8: with an 8-buffer x ring, slots
            # of tiles 8..15 are never reused, so GpSimd's ~11 us hold of
            # its x tile cannot block a later load.
            GPS_TILES = (8, 10)
            prod2_PK = spool.tile(
                [P, IN_SIZE], mybir.dt.float32, name="prod2_PK", tag="prod2"
            )
            for t in range(NQT, N_TILES - 2):
                load_x(t)
                if t in GPS_TILES:
                    nc.gpsimd.tensor_mul(prod2_PK[:], x_tiles[t][:], w_PK[:])
                    nc.scalar.activation(
                        prod2_PK[:],
                        prod2_PK[:],
                        mybir.ActivationFunctionType.Copy,
                        accum_out=acc_PT[:, t : t + 1],
                    )
                    continue
                nc.vector.affine_mul_reduce(
                    out=prod_PK[:],
                    accum_out=acc_PT[:, t : t + 1],
                    in0=x_tiles[t][:],
                    in1=w_PK[:],
                    scale=1.0,
                    bias=0.0,
                )

            # The last two tiles are split (loads AND compute) so the
            # final compute piece starts on the last 256 KiB rather than
            # the last 2 MiB. Segment s of tile t uses t's ring.
            def split_tile(t, segs, acc_off):
                x_PK = xpool.tile([P, IN_SIZE], mybir.dt.float32)
                x_tiles[t] = x_PK
                k0 = 0
                for s, seg_k in enumerate(segs):
                    seg = slice(k0, k0 + seg_k)
                    k0 += seg_k
                    ring(t).dma_start(out=x_PK[:, seg], in_=xt[t][:, seg])
                    nc.vector.affine_mul_reduce(
                        out=prod_PK[:, seg],
                        accum_out=acc_last[:, acc_off + s : acc_off + s + 1],
                        in0=x_PK[:, seg],
                        in1=w_PK[:, seg],
                        scale=1.0,
                        bias=0.0,
                    )

            t14, t15 = N_TILES - 2, N_TILES - 1
            split_tile(t14, (2048, 2048), 0)
            nc.vector.tensor_add(
                acc_PT[:, t14 : t14 + 1], acc_last[:, 0:1], acc_last[:, 1:2]
            )

            # Chunk A: tiles 0..14 go tanh -> transpose -> DMA as soon as
            # tile 14's combine lands, fully hidden under the tail of
            # the x stream. No DVE ops here — ScalarE/TensorE only.
            y_A = cpool.tile([P, N_A], mybir.dt.float32)
            nc.scalar.activation(
                y_A[:],
                acc_PT[:, 0:N_A],
                mybir.ActivationFunctionType.Tanh,
                bias=b_P1[:],
            )
            nc.scalar.activation(y_A[:], y_A[:], mybir.ActivationFunctionType.Tanh)
            yps_A = ppool.tile([N_A, P], mybir.dt.float32)
            nc.tensor.transpose(yps_A[:], y_A[:], ident[:])
            y_TA = cpool.tile([N_A, P], mybir.dt.float32)
            nc.scalar.copy(y_TA[:], yps_A[:])
            nc.scalar.dma_start(out=yv[0:N_A], in_=y_TA[:])

            # Tile 15: quarters then eighths, so the last affine covers
            # only 512 K-columns (256 KiB of x, ~0.65 us on DVE). Each
            # segment's partial goes to acc_last col 3 and is folded into
            # the col-2 running sum immediately (the fold adds are
            # emitted BETWEEN the affines so they execute in the DVE
            # idle gaps between segment arrivals); after the last affine
            # only one [128,1] add remains before the output chain.
            # name matches load_x's tiles so it shares their 8-slot ring
            # (untagged pool tiles are slotted by inferred variable name).
            x15 = xpool.tile([P, IN_SIZE], mybir.dt.float32, name="x_PK")
            k0 = 0
            for s, seg_k in enumerate((1024, 1024, 512, 512, 512, 512)):
                seg = slice(k0, k0 + seg_k)
                k0 += seg_k
                ring(t15).dma_start(out=x15[:, seg], in_=xt[t15][:, seg])
                col = 2 if s == 0 else 3
                nc.vector.affine_mul_reduce(
                    out=prod_PK[:, seg],
                    accum_out=acc_last[:, col : col + 1],
                    in0=x15[:, seg],
                    in1=w_PK[:, seg],
                    scale=1.0,
                    bias=0.0,
                )
                if 0 < s < 5:
                    nc.vector.tensor_add(
                        acc_last[:, 2:3], acc_last[:, 2:3], acc_last[:, 3:4]
                    )
            nc.vector.tensor_add(
                acc_PT[:, t15 : t15 + 1], acc_last[:, 2:3], acc_last[:, 3:4]
            )

            # Chunk B: only the 1-column tail. tanh(tanh(acc + b)) on
            # ScalarE (the DVE->ACT handoff needs no DVE drain), TensorE
            # transpose [128, 1] -> [1, 128], one 512 B DMA from the
            # scalar ring.
            y_B = cpool.tile([P, N_B], mybir.dt.float32)
            nc.scalar.activation(
                y_B[:],
                acc_PT[:, N_A:N_TILES],
                mybir.ActivationFunctionType.Tanh,
                bias=b_P1[:],
            )
            nc.scalar.activation(y_B[:], y_B[:], mybir.ActivationFunctionType.Tanh)
            yps_B = ppool.tile([N_B, P], mybir.dt.float32)
            nc.tensor.transpose(yps_B[:], y_B[:], ident[:])
            y_TB = cpool.tile([N_B, P], mybir.dt.float32)
            nc.scalar.copy(y_TB[:], yps_B[:])
            nc.scalar.dma_start(out=yv[N_A:N_TILES], in_=y_TB[:])
    nc.compile()
    return nc


def _get_nc():
    global _NC_CACHE
    if _NC_CACHE is None:
        _NC_CACHE = _build()
    return _NC_CACHE


def _run(x, w, b, **spmd_kwargs):
    """Shard, execute on 8 cores, gather. Returns (out, BassKernelResults)."""
    x = np.ascontiguousarray(np.asarray(x, dtype=np.float32))
    w = np.ascontiguousarray(np.asarray(w, dtype=np.float32))
    b = np.ascontiguousarray(np.asarray(b, dtype=np.float32))
    assert x.shape == (BATCH, IN_SIZE), x.shape

    nc = _get_nc()
    in_maps = [
        {"x": x[c * B_PER_CORE : (c + 1) * B_PER_CORE], "w": w, "b": b}
        for c in range(N_CORES)
    ]
    res = run_bass_kernel_spmd(nc, in_maps, list(range(N_CORES)), **spmd_kwargs)
    out = np.concatenate(
        [np.asarray(res.results[c]["y"]) for c in range(N_CORES)], axis=0
    )
    return out.astype(np.float32, copy=False), res


def kernel(x, w, b):
    try:
        out, _ = _run(x, w, b)
    except Exception:
        # Transient device-wedge (NRT_EXEC_UNIT_UNRECOVERABLE) has been
        # observed once on a first run and succeeded on retry.
        out, _ = _run(x, w, b)
    return out
